# revision 13
# baseline (speedup 1.0000x reference)
"""GCN encoder (Linear + 2x GCNConv + pair scatter-mean) on 8 Trainium2 cores.

Sharding: core c owns node rows [3750c, 3750(c+1)) and pair-target rows
[7500c, 7500(c+1)). Edges/pairs are sorted by destination on the host and
padded per 128-row destination block so all cores share one compiled program.
Aggregation = dma_gather of source rows + one-hot matmuls into PSUM per
destination block.

The pipeline is 4 collective-free bass NEFFs chained inside one jitted
shard_map, with jax.lax.all_gather exchanging the 30000x256 node tables
between stages (InstDMAGatherAnt and InstCollectiveCompute cannot coexist
in one NEFF on this runtime).
"""

import numpy as np

import concourse.bacc as bacc
import concourse.mybir as mybir
import concourse.tile as tile
from concourse.library_config import mlp
from concourse.masks import make_identity

NQ, NT, E, D_IN, D, NPAIR = 30000, 60000, 480000, 512, 256, 500000
NCORE = 8
NSLICE = NQ // NCORE  # 3750 nodes per core
NBLK = (NSLICE + 127) // 128  # 30 (last block 38 rows)
VSLICE = NT // NCORE  # 7500 pair-target rows per core
VBLK = (VSLICE + 127) // 128  # 59 (last block 76 rows)
G = 4  # gather chunk size in 128-row tiles (SWDGE desc-ring limit)

f32 = mybir.dt.float32
i16 = mybir.dt.int16


def _prep_streams(dst_all, src_all, slice_sz, nblk):
    """Sort (src,dst) pairs by dst, slice per core, pad each 128-row dst
    block to a tile count shared across cores."""
    order = np.argsort(dst_all, kind="stable")
    ds = dst_all[order]
    ss = src_all[order]
    bounds = np.array(
        [c * slice_sz + b * 128 for c in range(NCORE) for b in range(nblk)]
        + [NCORE * slice_sz],
        dtype=np.int64,
    )
    starts = np.searchsorted(ds, bounds[:-1])
    ends = np.append(starts[1:], len(ds))
    k = (ends - starts).reshape(NCORE, nblk)
    tb = np.maximum(1, -(-k.max(axis=0) // 128)).astype(int)
    L = int(tb.sum())
    gidx = np.zeros((NCORE, L * 128), np.int16)
    rix = np.full((NCORE, L * 128), -1.0, np.float32)
    off = 0
    for b in range(nblk):
        for c in range(NCORE):
            s0 = starts[c * nblk + b]
            n = k[c, b]
            gidx[c, off * 128 : off * 128 + n] = ss[s0 : s0 + n]
            rix[c, off * 128 : off * 128 + n] = (
                ds[s0 : s0 + n] - c * slice_sz - b * 128
            ).astype(np.float32)
        off += tb[b]
    idx16 = np.ascontiguousarray(
        np.tile(gidx.reshape(NCORE, L * 8, 16).transpose(0, 2, 1), (1, 8, 1))
    )
    idx32 = idx16.view(np.int32)  # packed pairs; device bitcasts back to i16
    rowwrap = np.ascontiguousarray(rix.reshape(NCORE, L, 128).transpose(0, 2, 1))
    return idx32, rowwrap, tb, L


def _wrap_cols(vec, slice_sz, nblk, fill):
    out = np.full((NCORE, nblk * 128), fill, np.float32)
    for c in range(NCORE):
        out[c, :slice_sz] = vec[c * slice_sz : (c + 1) * slice_sz]
    return np.ascontiguousarray(out.reshape(NCORE, nblk, 128).transpose(0, 2, 1))


def _new_nc():
    return bacc.Bacc(None, target_bir_lowering=False,
                     dynamic_dma_scratch_size=65536)


def _dinv_tile(nc, cp, wp, degw_t):
    dinv_t = cp.tile([128, NBLK], f32, tag="dinv", name="dinv")
    rec = wp.tile([128, NBLK], f32, tag="rec", name="rec")
    nc.vector.reciprocal(rec[:], degw_t[:])
    nc.scalar.sqrt(dinv_t[:], rec[:])
    return dinv_t


def _scale_transpose_out(nc, wp, psT, ident, dinv_t, y_sb, b, out_dram):
    """y_sb [feat 128(P) x (h*128+node)] -> node-major, x dinv -> DRAM rows."""
    m0 = b * 128
    m = min(128, NSLICE - m0)
    pt = psT.tile([128, D], f32, space="PSUM", tag="pst", name="pt")
    for h in range(2):
        nc.tensor.transpose(
            out=pt[:m, h * 128 : (h + 1) * 128],
            in_=y_sb[:, h * 128 : h * 128 + m],
            identity=ident[:],
        )
    trow = wp.tile([128, D], f32, tag="trow", name="trow")
    nc.vector.tensor_scalar(
        out=trow[:m],
        in0=pt[:m],
        scalar1=dinv_t[:m, b : b + 1],
        scalar2=None,
        op0=mybir.AluOpType.mult,
    )
    nc.sync.dma_start(out=out_dram[m0 : m0 + m, :], in_=trow[:m])


def _build_k1():
    """y1 = x_q @ (w_mlp@w1) + b_mlp@w1, rows scaled by dinv -> t1 slice."""
    nc = _new_nc()
    xqT = nc.dram_tensor("xqT", [D_IN, NSLICE], f32, kind="ExternalInput")
    wmT_h = nc.dram_tensor("wmT", [D, D_IN], f32, kind="ExternalInput")
    bm_h = nc.dram_tensor("bm", [D, 1], f32, kind="ExternalInput")
    w1_h = nc.dram_tensor("w1", [D, D], f32, kind="ExternalInput")
    degw_h = nc.dram_tensor("degw", [128, NBLK], f32, kind="ExternalInput")
    t1s = nc.dram_tensor("t1s", [NSLICE, D], f32, kind="ExternalOutput")

    with tile.TileContext(nc) as tc:
        with (
            tc.tile_pool(name="const", bufs=1) as cp,
            tc.tile_pool(name="work", bufs=3) as wp,
            tc.tile_pool(name="psY", bufs=2, space="PSUM") as psY,
            tc.tile_pool(name="psT", bufs=2, space="PSUM") as psT,
        ):
            degw_t = cp.tile([128, NBLK], f32, tag="degw", name="degw")
            nc.sync.dma_start(out=degw_t[:], in_=degw_h[:])
            w1_t, wmT_t, bm_t = [], [], []
            for kt in range(2):
                t = cp.tile([128, D], f32, tag=f"w1_{kt}", name=f"w1_{kt}")
                nc.sync.dma_start(out=t[:], in_=w1_h[kt * 128 : (kt + 1) * 128, :])
                w1_t.append(t)
                t = cp.tile([128, D_IN], f32, tag=f"wmT_{kt}", name=f"wmT_{kt}")
                nc.sync.dma_start(out=t[:], in_=wmT_h[kt * 128 : (kt + 1) * 128, :])
                wmT_t.append(t)
                t = cp.tile([128, 1], f32, tag=f"bm_{kt}", name=f"bm_{kt}")
                nc.sync.dma_start(out=t[:], in_=bm_h[kt * 128 : (kt + 1) * 128, :])
                bm_t.append(t)
            ident = cp.tile([128, 128], f32, tag="ident", name="ident")
            make_identity(nc, ident[:])
            dinv_t = _dinv_tile(nc, cp, wp, degw_t)

            weff = []
            for ib in range(4):
                pw = psY.tile([128, D], f32, space="PSUM", tag="psy", name="pw")
                for kt2 in range(2):
                    nc.tensor.matmul(
                        out=pw[:],
                        lhsT=wmT_t[kt2][:, ib * 128 : (ib + 1) * 128],
                        rhs=w1_t[kt2][:],
                        start=(kt2 == 0),
                        stop=(kt2 == 1),
                    )
                t = cp.tile([128, D], f32, tag=f"weff{ib}", name=f"weff{ib}")
                nc.scalar.copy(t[:], pw[:])
                weff.append(t)
            beff = []
            for h in range(2):
                pb = psT.tile([128, D], f32, space="PSUM", tag="pst", name="pb")
                for kt2 in range(2):
                    nc.tensor.matmul(
                        out=pb[:, :1],
                        lhsT=w1_t[kt2][:, h * 128 : (h + 1) * 128],
                        rhs=bm_t[kt2][:],
                        start=(kt2 == 0),
                        stop=(kt2 == 1),
                    )
                t = cp.tile([128, 1], f32, tag=f"beff{h}", name=f"beff{h}")
                nc.scalar.copy(t[:], pb[:, :1])
                beff.append(t)

            xq_t = []
            for kt in range(4):
                t = cp.tile([128, NSLICE], f32, tag=f"xq{kt}", name=f"xq{kt}")
                nc.sync.dma_start(out=t[:], in_=xqT[kt * 128 : (kt + 1) * 128, :])
                xq_t.append(t)

            for b in range(NBLK):
                m0 = b * 128
                m = min(128, NSLICE - m0)
                py = psY.tile([128, D], f32, space="PSUM", tag="psy", name="py")
                for h in range(2):
                    for kt in range(4):
                        nc.tensor.matmul(
                            out=py[:, h * 128 : h * 128 + m],
                            lhsT=weff[kt][:, h * 128 : (h + 1) * 128],
                            rhs=xq_t[kt][:, m0 : m0 + m],
                            start=(kt == 0),
                            stop=(kt == 3),
                            skip_group_check=(h == 1),
                        )
                y_sb = wp.tile([128, D], f32, tag="y_sb", name="y_sb")
                for h in range(2):
                    nc.scalar.activation(
                        y_sb[:, h * 128 : (h + 1) * 128],
                        py[:, h * 128 : (h + 1) * 128],
                        mybir.ActivationFunctionType.Identity,
                        bias=beff[h][:],
                    )
                _scale_transpose_out(nc, wp, psT, ident, dinv_t, y_sb, b, t1s)
    nc.finalize()
    return nc


def _emit_aggregate(nc, tc, cp, wp, gp, psA, table, idx_t, row_t, iota_t,
                    L_all, tb_list, nblk, epilogue):
    """One-hot scatter matmuls over dst blocks; epilogue(b, psum)."""
    gath_tiles, oh_tiles = {}, {}

    def ensure_chunk(ck):
        if ck in gath_tiles:
            return
        g = min(G, L_all - ck * G)
        gt = gp.tile([128, G * D], f32, tag="gath", name="gt")
        nc.gpsimd.dma_gather(
            gt[:, : g * D].rearrange("p (t e) -> p t e", e=D),
            table[:],
            idx_t[:, ck * G * 8 : ck * G * 8 + g * 8],
            g * 128,
            g * 128,
            D,
        )
        oh = gp.tile([128, G * 128], f32, tag="oh", name="oh")
        nc.vector.tensor_tensor(
            out=oh[:, : g * 128].rearrange("p (t i) -> p t i", i=128),
            in0=row_t[:, ck * G : ck * G + g].unsqueeze(2).to_broadcast(
                [128, g, 128]
            ),
            in1=iota_t[:].unsqueeze(1).to_broadcast([128, g, 128]),
            op=mybir.AluOpType.is_equal,
        )
        gath_tiles[ck] = gt
        oh_tiles[ck] = oh

    t_base = 0
    for b in range(nblk):
        pa = psA.tile([128, D], f32, space="PSUM", tag="psa", name="pa")
        for ti in range(tb_list[b]):
            t = t_base + ti
            ck, tl = t // G, t % G
            ensure_chunk(ck)
            nc.tensor.matmul(
                out=pa[:],
                lhsT=oh_tiles[ck][:, tl * 128 : (tl + 1) * 128],
                rhs=gath_tiles[ck][:, tl * D : (tl + 1) * D],
                start=(ti == 0),
                stop=(ti == tb_list[b] - 1),
            )
        t_base += tb_list[b]
        epilogue(b, pa)


def _emit_gcn_epilogue(nc, wp, dinv_t, brep_t, b, pa, out_cb):
    """x = elu(dinv*agg + b); hand node-major block to out_cb(b, m, x)."""
    m = min(128, NSLICE - b * 128)
    x = wp.tile([128, D], f32, tag="ep_x", name="ep_x")
    nc.scalar.activation(
        x[:],
        pa[:],
        mybir.ActivationFunctionType.Copy,
        bias=0.0,
        scale=dinv_t[:, b : b + 1],
    )
    nc.vector.tensor_tensor(out=x[:], in0=x[:], in1=brep_t[:],
                            op=mybir.AluOpType.add)
    e = wp.tile([128, D], f32, tag="ep_e", name="ep_e")
    nc.vector.tensor_scalar_min(e[:], x[:], 0.0)
    nc.scalar.activation(e[:], e[:], mybir.ActivationFunctionType.Exp)
    nc.vector.tensor_scalar_add(e[:], e[:], -1.0)
    nc.vector.tensor_tensor(out=x[:], in0=x[:], in1=e[:], op=mybir.AluOpType.max)
    out_cb(b, m, x)


def _agg_kernel_header(nc, tc, cp, L):
    """Common inputs for the aggregation kernels; returns loaded tiles."""
    tbl = nc.dram_tensor("tbl", [NQ, D], f32, kind="ExternalInput")
    # int16 is not a supported NEFF IO dtype through this jax path (XLA
    # inserts a convert to s32) -> ship packed int32, bitcast on load.
    eidx_h = nc.dram_tensor("eidx", [128, L * 4], mybir.dt.int32,
                            kind="ExternalInput")
    erow_h = nc.dram_tensor("erow", [128, L], f32, kind="ExternalInput")
    iota_h = nc.dram_tensor("iota", [128, 128], f32, kind="ExternalInput")
    nc.gpsimd.load_library(mlp)
    eidx_t = cp.tile([128, L * 8], i16, tag="eidx", name="eidx")
    nc.sync.dma_start(out=eidx_t[:], in_=eidx_h[:].bitcast(i16))
    erow_t = cp.tile([128, L], f32, tag="erow", name="erow")
    nc.sync.dma_start(out=erow_t[:], in_=erow_h[:])
    iota_t = cp.tile([128, 128], f32, tag="iota", name="iota")
    nc.sync.dma_start(out=iota_t[:], in_=iota_h[:])
    return tbl, eidx_t, erow_t, iota_t


def _build_k2(L, tb):
    """agg layer1 over T1 -> act1 -> y2 = act1 @ w2, x dinv -> t2 slice."""
    nc = _new_nc()
    with tile.TileContext(nc) as tc:
        with (
            tc.tile_pool(name="const", bufs=1) as cp,
            tc.tile_pool(name="work", bufs=3) as wp,
            tc.tile_pool(name="gath", bufs=2) as gp,
            tc.tile_pool(name="psA", bufs=2, space="PSUM") as psA,
            tc.tile_pool(name="psY", bufs=2, space="PSUM") as psY,
            tc.tile_pool(name="psT", bufs=2, space="PSUM") as psT,
        ):
            tbl, eidx_t, erow_t, iota_t = _agg_kernel_header(nc, tc, cp, L)
            w2_h = nc.dram_tensor("w2", [D, D], f32, kind="ExternalInput")
            b1r_h = nc.dram_tensor("b1r", [128, D], f32, kind="ExternalInput")
            degw_h = nc.dram_tensor("degw", [128, NBLK], f32, kind="ExternalInput")
            t2s = nc.dram_tensor("t2s", [NSLICE, D], f32, kind="ExternalOutput")
            w2_t = []
            for kt in range(2):
                t = cp.tile([128, D], f32, tag=f"w2_{kt}", name=f"w2_{kt}")
                nc.sync.dma_start(out=t[:], in_=w2_h[kt * 128 : (kt + 1) * 128, :])
                w2_t.append(t)
            b1r_t = cp.tile([128, D], f32, tag="b1r", name="b1r")
            nc.sync.dma_start(out=b1r_t[:], in_=b1r_h[:])
            degw_t = cp.tile([128, NBLK], f32, tag="degw", name="degw")
            nc.sync.dma_start(out=degw_t[:], in_=degw_h[:])
            ident = cp.tile([128, 128], f32, tag="ident", name="ident")
            make_identity(nc, ident[:])
            dinv_t = _dinv_tile(nc, cp, wp, degw_t)
            act1T = [
                cp.tile([128, NSLICE], f32, tag=f"act1T{h}", name=f"act1T{h}")
                for h in range(2)
            ]

            def to_act1T(b, m, x):
                pt2 = psT.tile([128, D], f32, space="PSUM", tag="pst", name="pt2")
                for h in range(2):
                    nc.tensor.transpose(
                        out=pt2[:, h * 128 : h * 128 + m],
                        in_=x[:m, h * 128 : (h + 1) * 128],
                        identity=ident[:m, :m],
                    )
                for h in range(2):
                    nc.scalar.copy(
                        act1T[h][:, b * 128 : b * 128 + m],
                        pt2[:, h * 128 : h * 128 + m],
                    )

            _emit_aggregate(
                nc, tc, cp, wp, gp, psA, tbl, eidx_t, erow_t, iota_t, L, tb,
                NBLK,
                lambda b, pa: _emit_gcn_epilogue(nc, wp, dinv_t, b1r_t, b, pa,
                                                 to_act1T),
            )

            for b in range(NBLK):
                m0 = b * 128
                m = min(128, NSLICE - m0)
                py = psY.tile([128, D], f32, space="PSUM", tag="psy", name="py")
                for h in range(2):
                    for kt in range(2):
                        nc.tensor.matmul(
                            out=py[:, h * 128 : h * 128 + m],
                            lhsT=w2_t[kt][:, h * 128 : (h + 1) * 128],
                            rhs=act1T[kt][:, m0 : m0 + m],
                            start=(kt == 0),
                            stop=(kt == 1),
                            skip_group_check=(h == 1),
                        )
                y_sb = wp.tile([128, D], f32, tag="y_sb", name="y_sb")
                nc.scalar.copy(y_sb[:], py[:])
                _scale_transpose_out(nc, wp, psT, ident, dinv_t, y_sb, b, t2s)
    nc.finalize()
    return nc


def _build_k3(L, tb):
    """agg layer2 over T2 -> h slice."""
    nc = _new_nc()
    with tile.TileContext(nc) as tc:
        with (
            tc.tile_pool(name="const", bufs=1) as cp,
            tc.tile_pool(name="work", bufs=3) as wp,
            tc.tile_pool(name="gath", bufs=2) as gp,
            tc.tile_pool(name="psA", bufs=2, space="PSUM") as psA,
        ):
            tbl, eidx_t, erow_t, iota_t = _agg_kernel_header(nc, tc, cp, L)
            b2r_h = nc.dram_tensor("b2r", [128, D], f32, kind="ExternalInput")
            degw_h = nc.dram_tensor("degw", [128, NBLK], f32, kind="ExternalInput")
            hs = nc.dram_tensor("hs", [NSLICE, D], f32, kind="ExternalOutput")
            b2r_t = cp.tile([128, D], f32, tag="b2r", name="b2r")
            nc.sync.dma_start(out=b2r_t[:], in_=b2r_h[:])
            degw_t = cp.tile([128, NBLK], f32, tag="degw", name="degw")
            nc.sync.dma_start(out=degw_t[:], in_=degw_h[:])
            dinv_t = _dinv_tile(nc, cp, wp, degw_t)

            def to_h(b, m, x):
                nc.sync.dma_start(out=hs[b * 128 : b * 128 + m, :], in_=x[:m])

            _emit_aggregate(
                nc, tc, cp, wp, gp, psA, tbl, eidx_t, erow_t, iota_t, L, tb,
                NBLK,
                lambda b, pa: _emit_gcn_epilogue(nc, wp, dinv_t, b2r_t, b, pa,
                                                 to_h),
            )
    nc.finalize()
    return nc


def _build_k4(LP, vtb):
    """pair scatter-mean over Th -> xt slice."""
    nc = _new_nc()
    with tile.TileContext(nc) as tc:
        with (
            tc.tile_pool(name="const", bufs=1) as cp,
            tc.tile_pool(name="work", bufs=3) as wp,
            tc.tile_pool(name="gath", bufs=2) as gp,
            tc.tile_pool(name="psA", bufs=2, space="PSUM") as psA,
        ):
            tbl, pidx_t, prow_t, iota_t = _agg_kernel_header(nc, tc, cp, LP)
            cntw_h = nc.dram_tensor("cntw", [128, VBLK], f32, kind="ExternalInput")
            xts = nc.dram_tensor("xts", [VSLICE, D], f32, kind="ExternalOutput")
            cntw_t = cp.tile([128, VBLK], f32, tag="cntw", name="cntw")
            nc.sync.dma_start(out=cntw_t[:], in_=cntw_h[:])
            vrec_t = cp.tile([128, VBLK], f32, tag="vrec", name="vrec")
            vtmp = wp.tile([128, VBLK], f32, tag="vtmp", name="vtmp")
            nc.vector.tensor_scalar_add(vtmp[:], cntw_t[:], 1.0)
            nc.vector.reciprocal(vrec_t[:], vtmp[:])

            def pair_epilogue(b, pa):
                m = min(128, VSLICE - b * 128)
                xt = wp.tile([128, D], f32, tag="ep_xt", name="ep_xt")
                nc.vector.tensor_scalar(
                    out=xt[:m],
                    in0=pa[:m],
                    scalar1=vrec_t[:m, b : b + 1],
                    scalar2=None,
                    op0=mybir.AluOpType.mult,
                )
                nc.sync.dma_start(out=xts[b * 128 : b * 128 + m, :], in_=xt[:m])

            _emit_aggregate(
                nc, tc, cp, wp, gp, psA, tbl, pidx_t, prow_t, iota_t, LP,
                vtb, VBLK, pair_epilogue,
            )
    nc.finalize()
    return nc


def _nc_io(nc):
    """(in_names, out_names, out_avals, partition_name) of a bass module."""
    import jax

    partition_name = nc.partition_id_tensor.name if nc.partition_id_tensor else None
    in_names, out_names, out_avals = [], [], []
    for alloc in nc.m.functions[0].allocations:
        if not isinstance(alloc, mybir.MemoryLocationSet):
            continue
        name = alloc.memorylocations[0].name
        if alloc.kind == "ExternalInput":
            if name != partition_name:
                in_names.append(name)
        elif alloc.kind == "ExternalOutput":
            out_names.append(name)
            out_avals.append(
                jax.core.ShapedArray(
                    tuple(alloc.tensor_shape), mybir.dt.np(alloc.dtype)
                )
            )
    return in_names, out_names, out_avals, partition_name


_CACHE = {}


def _stage_fn(nc, rep_names):
    """Jitted shard_map callable for one bass module. Inputs in rep_names are
    replicated (P()); everything else (incl. output zero-buffers) is sharded
    along "core". Returns (fn, in_names, out_names)."""
    import jax
    import numpy as np_
    from jax.experimental.shard_map import shard_map
    from jax.sharding import Mesh, PartitionSpec

    from concourse import bass2jax

    in_names, out_names, out_avals, partition_name = _nc_io(nc)
    all_in = list(in_names) + list(out_names)
    if partition_name is not None:
        all_in_full = all_in + [partition_name]
    else:
        all_in_full = all_in

    def body(*args):
        operands = list(args)
        if partition_name is not None:
            operands.append(bass2jax.partition_id_tensor())
        outs = bass2jax._bass_exec_p.bind(
            *operands,
            out_avals=tuple(out_avals),
            in_names=tuple(all_in_full),
            out_names=tuple(out_names),
            lowering_input_output_aliases=(),
            sim_require_finite=True,
            sim_require_nnan=True,
            nc=nc,
        )
        return tuple(outs)

    mesh = _CACHE["mesh"]
    in_specs = tuple(
        PartitionSpec() if nm in rep_names else PartitionSpec("core")
        for nm in all_in
    )
    out_specs = (PartitionSpec("core"),) * len(out_names)
    n_outs = len(out_names)
    donate = tuple(range(len(in_names), len(in_names) + n_outs))
    fn = jax.jit(
        shard_map(body, mesh=mesh, in_specs=in_specs, out_specs=out_specs,
                  check_rep=False),
        keep_unused=True,
        donate_argnums=donate,
    )
    return fn, in_names, out_names, out_avals


def _ag_fn():
    import jax
    from jax.experimental.shard_map import shard_map
    from jax.sharding import PartitionSpec

    mesh = _CACHE["mesh"]
    return jax.jit(
        shard_map(
            lambda x: jax.lax.all_gather(x, "core", tiled=True),
            mesh=mesh,
            in_specs=(PartitionSpec("core"),),
            out_specs=PartitionSpec(),
            check_rep=False,
        )
    )


def _get_program():
    if "stages" in _CACHE:
        return _CACHE["stages"]
    import jax
    import numpy as np_
    from jax.sharding import Mesh

    from concourse import bass2jax

    bass2jax.install_neuronx_cc_hook()
    devices = jax.devices()[:NCORE]
    _CACHE["mesh"] = Mesh(np_.asarray(devices), ("core",))
    L, tb, LP, vtb = _CACHE["shapes"]
    rep = {"wmT", "bm", "w1", "w2", "b1r", "b2r", "iota", "tbl"}
    s1 = _stage_fn(_build_k1(), rep)
    s2 = _stage_fn(_build_k2(L, tb), rep)
    s3 = _stage_fn(_build_k3(L, tb), rep)
    s4 = _stage_fn(_build_k4(LP, vtb), rep)
    _CACHE["stages"] = (s1, s2, s3, s4, _ag_fn())
    return _CACHE["stages"]


def _run_stage(stage, rep_names, feed):
    """Call one stage: feed dict name->array; appends fresh zero outputs."""
    import jax
    import numpy as np_
    from jax.sharding import NamedSharding, PartitionSpec

    fn, in_names, out_names, out_avals = stage
    mesh = _CACHE["mesh"]
    sh_core = NamedSharding(mesh, PartitionSpec("core"))
    args = [feed[nm] for nm in in_names]
    for av in out_avals:
        args.append(
            jax.device_put(
                np_.zeros((NCORE * av.shape[0], *av.shape[1:]), av.dtype), sh_core
            )
        )
    outs = fn(*args)
    return dict(zip(out_names, outs))


def kernel(x_q, w_mlp, b_mlp, w1, b1, w2, b2, edge_index, cs_u, cs_v, nt):
    x_q = np.asarray(x_q, np.float32)
    w_mlp = np.asarray(w_mlp, np.float32)
    b_mlp = np.asarray(b_mlp, np.float32)
    w1 = np.asarray(w1, np.float32)
    b1 = np.asarray(b1, np.float32)
    w2 = np.asarray(w2, np.float32)
    b2 = np.asarray(b2, np.float32)
    edge_index = np.asarray(edge_index, np.int64)
    cs_u = np.asarray(cs_u, np.int64)
    cs_v = np.asarray(cs_v, np.int64)
    assert int(nt) == NT and x_q.shape == (NQ, D_IN)

    src, dst = edge_index[0], edge_index[1]
    loop = np.arange(NQ, dtype=np.int64)
    s_all = np.concatenate([src, loop])
    d_all = np.concatenate([dst, loop])

    eidx, erow, tb, L = _prep_streams(d_all, s_all, NSLICE, NBLK)
    pidx, prow, vtb, LP = _prep_streams(cs_v, cs_u, VSLICE, VBLK)

    deg = np.bincount(d_all, minlength=NQ).astype(np.float32)
    cnt = np.bincount(cs_v, minlength=NT).astype(np.float32)
    degw = _wrap_cols(deg, NSLICE, NBLK, 1.0)
    cntw = _wrap_cols(cnt, VSLICE, VBLK, 0.0)

    iota = np.ascontiguousarray(
        np.tile(np.arange(128, dtype=np.float32)[None, :], (128, 1))
    )
    b1r = np.ascontiguousarray(np.tile(b1[None, :], (128, 1)).astype(np.float32))
    b2r = np.ascontiguousarray(np.tile(b2[None, :], (128, 1)).astype(np.float32))
    wmT = np.ascontiguousarray(w_mlp.T)
    bm = np.ascontiguousarray(b_mlp.reshape(D, 1))

    _CACHE["shapes"] = (L, list(tb), LP, list(vtb))
    s1, s2, s3, s4, ag = _get_program()

    import jax
    from jax.sharding import NamedSharding, PartitionSpec

    mesh = _CACHE["mesh"]
    sh_core = NamedSharding(mesh, PartitionSpec("core"))
    sh_rep = NamedSharding(mesh, PartitionSpec())

    def put_core(per_core):
        return jax.device_put(np.concatenate(per_core, axis=0), sh_core)

    def put_rep(a):
        return jax.device_put(a, sh_rep)

    feed = {
        "xqT": put_core(
            [np.ascontiguousarray(x_q[c * NSLICE : (c + 1) * NSLICE].T)
             for c in range(NCORE)]
        ),
        "wmT": put_rep(wmT),
        "bm": put_rep(bm),
        "w1": put_rep(w1),
        "w2": put_rep(w2),
        "b1r": put_rep(b1r),
        "b2r": put_rep(b2r),
        "iota": put_rep(iota),
        "degw": put_core(list(degw)),
        "cntw_core": put_core(list(cntw)),
        "eidx": put_core(list(eidx)),
        "erow": put_core(list(erow)),
        "pidx": put_core(list(pidx)),
        "prow": put_core(list(prow)),
    }

    def pipeline():
        t1s = _run_stage(s1, None, feed)["t1s"]
        T1 = ag(t1s)
        t2s = _run_stage(s2, None, {**feed, "tbl": T1})["t2s"]
        T2 = ag(t2s)
        hs = _run_stage(s3, None, {**feed, "tbl": T2})["hs"]
        Th = ag(hs)
        f4 = {**feed, "tbl": Th, "eidx": feed["pidx"], "erow": feed["prow"],
              "cntw": feed["cntw_core"]}
        xts = _run_stage(s4, None, f4)["xts"]
        return hs, xts

    hs, xts = pipeline()
    kernel.last = pipeline
    h = np.asarray(hs).reshape(NQ, D)
    x_t = np.asarray(xts).reshape(NT, D)
    return h, x_t
